# revision 56
# baseline (speedup 1.0000x reference)
"""BatchMultiHeadGraphAttention kernel for TRN2 (8 NeuronCores).

Reference computation (per graph b):
  h_prime = h @ w[head]                 [n, fo] per head
  t = tanh(h_prime)
  src[n] = t @ a_src[head];  dst[n] = t @ a_dst[head]
  s[i, j] = leaky_relu(src[i] + dst[j], 0.2)
  s masked where ~(adj | I); softmax over j; out = p @ h_prime + bias

Sharding: data-parallel over batch - one graph per core (BS=8, 8 cores).

Hybrid per-(head, jb) tile paths, scores transposed (p[j, i]) so the
output matmul contracts over the partition axis:
  - A-path (ACT-heavy, 4 tiles/head): PE matmuls build
    s = dst_j + src_i - 144*invalid in PSUM (two k=1 outer products from
    single-row scatter tiles + an identity matmul folding the additive
    mask), then ACT Prelu + ACT Exp. exp(s - 144) ~ 0 masks invalid edges.
  - V-path (DVE-heavy, 4 tiles/head): rank-1 factorization of the
    reference nonlinearity,
       exp(leaky(x)) = max(exp(x), exp(0.2 x)) = max(E_i*F_j, E~_i*F~_j)
    with E=exp(src), F=exp(dst), E~=exp(0.2 src), F~=exp(0.2 dst): two
    DVE tensor_scalar 4x passes (replicated E rows x per-partition F
    scalars), a DVE tensor max, and a multiplicative {0,1} mask - no N*N
    exp, no PE score streams, no PSUM. Pool/GPSIMD has no ALU ops on TRN2
    (DMA/copy/memset only), so these stay on DVE; the A/V split balances
    ACT vs DVE occupancy (~84 us each).
  - src/dst rows per head pair via one [8, N] PE matmul against weight
    columns (src, dst, and 0.2-scaled copies), one ACT Exp gives all of
    E/F/E~/F~; rows go through a DRAM scratch and come back as
    partition-replicated [128, 2, N] tiles via HWDGE broadcast loads
    (stride-0 leading dim). Per-partition F columns via 8-row PE
    transposes. Raw src/dst rows are DMA-scattered into per-pair
    single-row tiles (no init needed, no cross-pair WAW serialization).
  - adj loads via casting SWDGE DMAs (u8 -> bf16 {0,1}, 8x128
    descriptors to keep the 1024-desc SWDGE ring fluid), DMA-transposed
    on the sync HWDGE queue interleaved with the pair-loop stores so the
    in-order SEQ reaches each as its data lands; diag forced valid with
    an identity max; additive A-mask derived as 144v-144.
  - bias folded into hp via an extra k=1 PE matmul (ones x bias-row);
    softmax normalization makes (sum p*(hp+bias))/sum p = out + bias
    exact; the hp ones-column yields the softmax denominator inside the
    output matmul; out stores via HWDGE on the SP queue.
  - phase B software-pipelined one head deep; emission order is tuned
    against the in-order per-engine SEQ FIFOs (products -> maxes ->
    masks batches; pair heads/tails staggered).
"""

import sys

import numpy as np

try:
    import concourse.bass  # noqa: F401
except ImportError:
    sys.path.insert(0, "/opt/trn_rl_repo")

BS, N, H, FI, FO = 8, 1024, 8, 256, 64
P = 128
NB = N // P     # 8 node blocks
FC = FI // P    # 2 f_in chunks
HP = H // 2     # head pairs
ALPHA = 0.2
BIG = 144.0

# per-head A-path jb positions (4 per head: 32 A-tiles / 32 V-tiles).
# Pool has no ALU ops on TRN2 (DMA/copy/memset only), so all V-path
# max/min ops run on DVE; the A/V split balances ACT vs DVE.
_A_COUNT = [4, 4, 4, 4, 4, 4, 4, 4]
A_JB = {
    hd: set(((hd + t) % NB) for t in range(_A_COUNT[hd]))
    for hd in range(H)
}
def _v_eng(kv):
    return "D", "D"


def build_bass():
    import concourse.bass as bass
    import concourse.mybir as mybir
    from concourse import bacc
    from concourse.masks import make_identity
    from concourse.tile import TileContext

    f32 = mybir.dt.float32
    bf16 = mybir.dt.bfloat16
    u8 = mybir.dt.uint8
    AF = mybir.ActivationFunctionType
    OP = mybir.AluOpType

    nc = bacc.Bacc(trn_type="TRN2")

    h_d = nc.dram_tensor("h", [N, FI], f32, kind="ExternalInput")
    adj_d = nc.dram_tensor("adj", [N, N], u8, kind="ExternalInput")
    w_d = nc.dram_tensor("w", [H, FI, FO], f32, kind="ExternalInput")
    asrc_d = nc.dram_tensor("a_src", [H, FO], f32, kind="ExternalInput")
    adst_d = nc.dram_tensor("a_dst", [H, FO], f32, kind="ExternalInput")
    bias_d = nc.dram_tensor("bias", [FO], f32, kind="ExternalInput")
    out_d = nc.dram_tensor("out", [H, N, FO], f32, kind="ExternalOutput")
    # E/E~/F/F~ rows staged through DRAM for partition-replication loads
    scr_d = nc.dram_tensor("scr", [8 * HP, N], bf16, kind="Internal")

    with TileContext(nc) as tc:
        with (
            tc.tile_pool(name="singles", bufs=1) as singles,
            tc.tile_pool(name="temps", bufs=2) as temps,
            tc.tile_pool(name="ppool", bufs=2) as ppool,
            tc.tile_pool(name="spool", bufs=2) as spool,
            tc.tile_pool(name="epool", bufs=4) as epool,
        ):
            # ---------- long-lived tiles ----------
            ident = singles.tile([P, P], f32)
            make_identity(nc, ident)
            identb = singles.tile([P, P], bf16)
            nc.gpsimd.tensor_copy(out=identb, in_=ident)

            hT_sb = singles.tile([P, FC, N], bf16)  # [f_local, fc, n]
            # A-path score operands: per-pair single-row tiles (scatters
            # fully overwrite them - no init, no WAW serialization). Scores
            # use two k=1 matmuls (dst x ones + ones x src) + mask fold.
            dstp, srcp = [], []
            for q in range(HP):
                dstp.append(singles.tile([1, 2, N], bf16, name=f"dstp{q}"))
                srcp.append(singles.tile([1, 2, N], bf16, name=f"srcp{q}"))
            ones_row = singles.tile([1, P], bf16)
            nc.vector.memset(ones_row, 1.0)
            ones1k = singles.tile([1, N], bf16)
            nc.vector.memset(ones1k, 1.0)

            # hp with ones column (col FO); bias folded in via PE matmul
            hp_all = singles.tile([P, NB, H, FO + 1], bf16)
            nc.vector.memset(hp_all[:, :, :, FO : FO + 1], 1.0)

            # mask forms [j_local, jb, i]: vT {0,1} multiplies V-tiles
            # directly; maskAdd = 144v-144 is the A-path additive form
            vT = singles.tile([P, NB, N], bf16)
            maskAdd = singles.tile([P, NB, N], bf16)

            # V-path replicated rows: rotating pool of [P, {E,E~}, N]
            # per-partition scalars: [j_local, jb, pair, row8] (f32)
            cols = singles.tile([P, NB, HP, 8], f32)
            erep_t = {}

            # ================= phase A: prep =================
            with (
                tc.tile_pool(name="phA", bufs=1) as phA,
                tc.tile_pool(name="tempA", bufs=4) as tempA,
                tc.tile_pool(name="psumA", bufs=2, space="PSUM") as psA,
                tc.tile_pool(name="psumHP", bufs=1, space="PSUM") as psHP,
                tc.tile_pool(name="psumR", bufs=2, space="PSUM") as psR,
                tc.tile_pool(name="psumD", bufs=1, space="PSUM") as psD,
                tc.tile_pool(name="psumC", bufs=1, space="PSUM") as psC,
            ):
                # w and h first (HWDGE f32 loads - no SWDGE ring space,
                # hardware desc-gen) + on-chip bf16 converts; they gate the
                # tanh/psd chain everything else hangs off
                w_f = phA.tile([P, FC, H, FO], f32)
                w_re = w_d.rearrange("h (c p) o -> p c h o", p=P)
                w_sb = phA.tile([P, FC, H, FO], bf16)
                h_sb = phA.tile([P, NB, FI], f32)
                h_re = h_d.rearrange("(nb p) f -> p nb f", p=P)
                nc.sync.dma_start(out=w_f[:, 0], in_=w_re[:, 0])
                nc.sync.dma_start(out=h_sb[:, 0:4], in_=h_re[:, 0:4])
                nc.vector.tensor_copy(out=w_sb[:, 0], in_=w_f[:, 0])
                nc.sync.dma_start(out=w_f[:, 1], in_=w_re[:, 1])
                nc.sync.dma_start(out=h_sb[:, 4:8], in_=h_re[:, 4:8])
                nc.vector.tensor_copy(out=w_sb[:, 1], in_=w_f[:, 1])
                # adjacency: casting SWDGE DMA u8 -> bf16 {0.0, 1.0};
                # DMA-transposes follow immediately on the scalar HWDGE
                # queue (keeps the sync queue free for scr/erep/out traffic)
                adjb = phA.tile([P, NB, N], bf16)
                adj_re = adj_d.rearrange("(ib p) j -> p ib j", p=P)
                for ib in range(NB):
                    nc.gpsimd.dma_start(
                        out=adjb[:, ib : ib + 1],
                        in_=adj_re[:, ib : ib + 1],
                    )
                for ib in range(NB):
                    nc.sync.dma_start_transpose(
                        out=vT[:, :, ib * P : (ib + 1) * P],
                        in_=adjb[:, ib],
                    )

                # a_src/a_dst: load as [16, 64], transpose to [64, 16]
                a2d = phA.tile([2 * H, FO], f32)
                nc.sync.dma_start(out=a2d[0:H], in_=asrc_d[:, :])
                nc.sync.dma_start(out=a2d[H : 2 * H], in_=adst_d[:, :])

                # bias row replicated over heads, bf16, with 0 in col FO
                bias_f = phA.tile([1, FO], f32)
                nc.sync.dma_start(
                    out=bias_f,
                    in_=bass.AP(tensor=bias_d, offset=0, ap=[[0, 1], [1, FO]]),
                )
                bias_hrow = phA.tile([1, H, FO + 1], bf16)
                nc.vector.memset(bias_hrow, 0.0)
                nc.vector.tensor_copy(
                    out=bias_hrow[:, :, 0:FO],
                    in_=bass.AP(
                        tensor=bias_f.tensor,
                        offset=bias_f.offset,
                        ap=[list(bias_f.ap[0]), [0, H], [1, FO]],
                    ),
                )

                # a2T: [64, 16] weight columns; a2p8 adds 0.2-scaled copies
                pa2 = psR.tile([FO, 2 * H], f32, tag="hpT")
                nc.tensor.transpose(pa2, a2d, ident[0 : 2 * H, 0 : 2 * H])
                a2T = phA.tile([FO, 2 * H], bf16)
                nc.vector.tensor_copy(out=a2T, in_=pa2)
                # paired-head weight cols: a2p8[:, m, q]; head 2q on parts
                # 0-63, head 2q+1 on 64-127; m = (src, dst, src', dst',
                # .2src, .2dst, .2src', .2dst')
                a2p8 = phA.tile([P, 8, HP], bf16)
                nc.vector.memset(a2p8, 0.0)
                for q in range(HP):
                    nc.vector.tensor_copy(
                        out=a2p8[0:FO, 0:2, q],
                        in_=bass.AP(
                            tensor=a2T.tensor, offset=a2T.offset + 2 * q,
                            ap=[list(a2T.ap[0]), [H, 2]],
                        ),
                    )
                    nc.vector.tensor_copy(
                        out=a2p8[FO : 2 * FO, 2:4, q],
                        in_=bass.AP(
                            tensor=a2T.tensor, offset=a2T.offset + 2 * q + 1,
                            ap=[list(a2T.ap[0]), [H, 2]],
                        ),
                    )
                nc.vector.tensor_scalar(
                    out=a2p8[:, 4:8, :], in0=a2p8[:, 0:4, :],
                    scalar1=ALPHA, scalar2=None, op0=OP.mult,
                )

                # hT: h transposed to [f_local, fc, n]
                for nb in range(NB):
                    for fc in range(FC):
                        pt = psA.tile([P, P], f32, tag="tr")
                        nc.tensor.transpose(
                            pt, h_sb[:, nb, fc * P : (fc + 1) * P], ident
                        )
                        nc.scalar.activation(
                            out=hT_sb[:, fc, nb * P : (nb + 1) * P],
                            in_=pt, func=AF.Copy,
                        )

                # ----- src/dst rows via paired transposed h_prime -----
                # psd8 rows: (src2q, dst2q, src2q+1, dst2q+1) then 0.2x
                # copies; one ACT Exp per pair gives (E,F,E',F',E~,...)
                ptc = psC.tile([P, NB, HP, 8], bf16, tag="cols")
                st8s = {}

                def emit_diag(ib):
                    nc.vector.tensor_max(
                        out=vT[:, ib, ib * P : (ib + 1) * P],
                        in0=vT[:, ib, ib * P : (ib + 1) * P],
                        in1=identb,
                    )

                def emit_derives(half):
                    hs = slice(half * 512, (half + 1) * 512)
                    for jb in range(NB):
                        nc.vector.tensor_scalar(
                            out=maskAdd[:, jb, hs], in0=vT[:, jb, hs],
                            scalar1=BIG, scalar2=-BIG,
                            op0=OP.mult, op1=OP.add,
                        )

                def emit_pair_head(q):
                    # tanh chain -> psd8 -> SBUF st8; the DVE copy alone
                    # frees the PSUM tile so pairs advance at tanh rate
                    tTp = tempA.tile([P, N], bf16, tag="tT")
                    st8 = phA.tile([8, N], bf16, tag=f"st8{q}")
                    for half in range(2):
                        hs = slice(half * 512, (half + 1) * 512)
                        phT = psR.tile([P, 512], f32, tag="hpT")
                        for fc in range(FC):
                            nc.tensor.matmul(
                                phT,
                                lhsT=w_sb[:, fc, 2 * q : 2 * q + 2, :],
                                rhs=hT_sb[:, fc, hs],
                                start=(fc == 0),
                                stop=(fc == FC - 1),
                                skip_group_check=True,
                            )
                        nc.scalar.activation(
                            out=tTp[:, hs], in_=phT, func=AF.Tanh
                        )
                        psd8 = psD.tile([8, 512], f32, tag="sd")
                        nc.tensor.matmul(
                            psd8, lhsT=a2p8[:, :, q], rhs=tTp[:, hs],
                            start=True, stop=True,
                        )
                        nc.scalar.activation(
                            out=st8[:, hs], in_=psd8, func=AF.Copy
                        )
                    st8s[q] = st8

                def emit_pair_tail(q):
                    # rows: (src2q, dst2q, src2q+1, dst2q+1, 0.2x those)
                    st8 = st8s[q]
                    eAll = phA.tile([8, N], bf16, tag=f"eAll{q}")
                    nc.scalar.activation(out=eAll, in_=st8, func=AF.Exp)
                    # to DRAM scratch for broadcast-loads
                    nc.sync.dma_start(out=scr_d[8 * q : 8 * q + 8, :], in_=eAll)
                    # per-partition scalar columns via PE transposes
                    for jb in range(NB):
                        nc.tensor.transpose(
                            ptc[:, jb, q, :],
                            eAll[:, jb * P : (jb + 1) * P],
                            identb[0:8, 0:8],
                        )
                    nc.vector.tensor_copy(
                        out=cols[:, :, q, :], in_=ptc[:, :, q, :]
                    )
                    # broadcast-load E/E~ replicated rows for both heads
                    for r in range(2):
                        hd = 2 * q + r
                        et = epool.tile([P, 2, N], bf16, tag="erep")
                        nc.sync.dma_start(
                            out=et,
                            in_=bass.AP(
                                tensor=scr_d,
                                offset=(8 * q + 2 * r) * N,
                                ap=[[0, P], [4 * N, 2], [1, N]],
                            ),
                        )
                        erep_t[hd] = et
                    # scatter raw rows into A-path operands:
                    # dsts (st8 rows 1,3) -> dstp[q], srcs (0,2) -> srcp[q]
                    nc.gpsimd.dma_start(
                        out=dstp[q][0:1, :, :],
                        in_=bass.AP(
                            tensor=st8.tensor, offset=st8.offset + N,
                            ap=[[2 * N, 2], [1, N]],
                        ),
                    )
                    nc.gpsimd.dma_start(
                        out=srcp[q][0:1, :, :],
                        in_=bass.AP(
                            tensor=st8.tensor, offset=st8.offset,
                            ap=[[2 * N, 2], [1, N]],
                        ),
                    )
                for q in range(HP):
                    emit_pair_head(q)
                    if q >= 1:
                        emit_pair_tail(q - 1)
                emit_pair_tail(HP - 1)
                # adj diag forcing + additive-mask derivation AFTER the pair
                # loop: keeps the DVE FIFO clear of transpose-gated work
                # while the pair chain drains
                for ib in range(NB):
                    emit_diag(ib)
                for half in range(2):
                    emit_derives(half)

                # ----- hp (+ones col, bias via extra matmul) -----
                for nb in range(NB):
                    php = psHP.tile([P, H, FO], f32, tag="hp")
                    nc.tensor.matmul(
                        php,
                        lhsT=ones_row,
                        rhs=bias_hrow[:, :, 0:FO],
                        start=True, stop=False,
                    )
                    for fc in range(FC):
                        nc.tensor.matmul(
                            php,
                            lhsT=hT_sb[:, fc, nb * P : (nb + 1) * P],
                            rhs=w_sb[:, fc],
                            start=False,
                            stop=(fc == FC - 1),
                        )
                    nc.vector.tensor_copy(out=hp_all[:, nb, :, 0:FO], in_=php)

            # ================= phase B: attention =================
            with (
                tc.tile_pool(name="psumS", bufs=3, space="PSUM") as psS,
                tc.tile_pool(name="psumO", bufs=1, space="PSUM") as psO,
            ):
                def emit_scores(hd):
                    q, r = hd // 2, hd % 2
                    p_sb = ppool.tile([P, NB, N], bf16, tag="p")
                    a_jbs = [jb for jb in range(NB) if jb in A_JB[hd]]
                    v_jbs = [jb for jb in range(NB) if jb not in A_JB[hd]]
                    # A-path first: PE matmuls + ACT prelu/exp
                    for jb in a_jbs:
                        ps = psS.tile([P, N], f32, tag="spre")
                        for half in range(2):
                            hs = slice(half * 512, (half + 1) * 512)
                            nc.tensor.matmul(
                                ps[:, hs],
                                lhsT=dstp[q][:, r, jb * P : (jb + 1) * P],
                                rhs=ones1k[:, hs],
                                start=True,
                                stop=False,
                                skip_group_check=True,
                            )
                            nc.tensor.matmul(
                                ps[:, hs],
                                lhsT=ones_row,
                                rhs=srcp[q][:, r, hs],
                                start=False,
                                stop=False,
                                skip_group_check=True,
                            )
                            nc.tensor.matmul(
                                ps[:, hs],
                                lhsT=identb,
                                rhs=maskAdd[:, jb, hs],
                                start=False,
                                stop=True,
                                skip_group_check=True,
                            )
                        s_t = spool.tile([P, N], bf16, tag="st")
                        nc.scalar.activation(
                            out=s_t, in_=ps, func=AF.Prelu, alpha=ALPHA
                        )
                        nc.scalar.activation(
                            out=p_sb[:, jb, :], in_=s_t, func=AF.Exp
                        )
                    # V-path: g = E_rep*F_j, g~ = E~_rep*F~_j, max, min-mask
                    # batched: all products, then maxes, then masks, so the
                    # in-order DVE queue never head-of-line blocks on Pool
                    gs, gts, us = {}, {}, {}
                    for jb in v_jbs:
                        g = spool.tile([P, N], bf16, tag="g")
                        nc.vector.tensor_scalar(
                            out=g, in0=erep_t[hd][:, 0, :],
                            scalar1=cols[:, jb, q, 1 + 2 * r : 2 + 2 * r],
                            scalar2=None, op0=OP.mult,
                        )
                        gt = spool.tile([P, N], bf16, tag="gt")
                        nc.vector.tensor_scalar(
                            out=gt, in0=erep_t[hd][:, 1, :],
                            scalar1=cols[:, jb, q, 5 + 2 * r : 6 + 2 * r],
                            scalar2=None, op0=OP.mult,
                        )
                        gs[jb], gts[jb] = g, gt
                    for jb in v_jbs:
                        u = spool.tile([P, N], bf16, tag="u")
                        nc.vector.tensor_max(out=u, in0=gs[jb], in1=gts[jb])
                        us[jb] = u
                    for jb in v_jbs:
                        nc.vector.tensor_mul(
                            out=p_sb[:, jb, :], in0=us[jb],
                            in1=vT[:, jb, :],
                        )
                    return p_sb

                def emit_out(hd, p_sb):
                    out_re = out_d[hd].rearrange("(p ic) o -> p ic o", ic=NB)
                    rz = temps.tile([P, NB, 1], f32, tag="rz")
                    o_sb = temps.tile([P, NB, FO], f32, tag="osb")
                    for grp in range(2):
                        po = psO.tile([P, 4, FO + 1], f32,
                                      tag="o2a" if grp == 0 else "o2b")
                        ics = range(4 * grp, 4 * grp + 4)
                        for ic in ics:
                            icl = ic % 4
                            for jb in range(NB):
                                lhsT_str = bass.AP(
                                    tensor=p_sb.tensor,
                                    offset=p_sb[:, jb, ic : ic + 1].offset,
                                    ap=[list(p_sb.ap[0]), [NB, P]],
                                )
                                nc.tensor.matmul(
                                    po[:, icl, :],
                                    lhsT=lhsT_str,
                                    rhs=hp_all[:, jb, hd, :],
                                    start=(jb == 0),
                                    stop=(jb == NB - 1),
                                )
                        gs = slice(4 * grp, 4 * grp + 4)
                        nc.vector.reciprocal(
                            out=rz[:, gs], in_=po[:, :, FO : FO + 1]
                        )
                        rzb = bass.AP(
                            tensor=rz.tensor, offset=rz[:, gs, :].offset,
                            ap=[list(rz.ap[0]), [rz.ap[1][0], 4], [0, FO]],
                        )
                        nc.vector.tensor_mul(
                            out=o_sb[:, gs, :], in0=po[:, :, 0:FO], in1=rzb
                        )
                        nc.sync.dma_start(
                            out=out_re[:, gs], in_=o_sb[:, gs]
                        )

                prev_p = None
                for hd in range(H):
                    p_new = emit_scores(hd)
                    if prev_p is not None:
                        emit_out(hd - 1, prev_p)
                    prev_p = p_new
                emit_out(H - 1, prev_p)
    nc.finalize()
    return nc


_NC_CACHE = None
TRACE = False
LAST_RESULT = None


def kernel(h, adj, w, a_src, a_dst, bias):
    global _NC_CACHE
    from concourse.bass_utils import run_bass_kernel_spmd

    if _NC_CACHE is None:
        _NC_CACHE = build_bass()
    nc = _NC_CACHE

    h = np.ascontiguousarray(np.asarray(h, dtype=np.float32))
    adj_u8 = np.ascontiguousarray(np.asarray(adj).astype(np.uint8))
    w = np.ascontiguousarray(np.asarray(w, dtype=np.float32))
    a_src2 = np.ascontiguousarray(np.asarray(a_src, dtype=np.float32)[..., 0])
    a_dst2 = np.ascontiguousarray(np.asarray(a_dst, dtype=np.float32)[..., 0])
    bias = np.ascontiguousarray(np.asarray(bias, dtype=np.float32))

    in_maps = [
        {
            "h": h[b],
            "adj": adj_u8[b],
            "w": w,
            "a_src": a_src2,
            "a_dst": a_dst2,
            "bias": bias,
        }
        for b in range(BS)
    ]
    res = run_bass_kernel_spmd(
        nc, in_maps, core_ids=list(range(BS)), trace=TRACE,
        trace_cores=list(range(BS)) if TRACE else None,
    )
    if TRACE:
        global LAST_RESULT
        LAST_RESULT = res
    out = np.stack([r["out"] for r in res.results], axis=0)
    return out.astype(np.float32)


# revision 57
# speedup vs baseline: 1.0133x; 1.0133x over previous
"""BatchMultiHeadGraphAttention kernel for TRN2 (8 NeuronCores).

Reference computation (per graph b):
  h_prime = h @ w[head]                 [n, fo] per head
  t = tanh(h_prime)
  src[n] = t @ a_src[head];  dst[n] = t @ a_dst[head]
  s[i, j] = leaky_relu(src[i] + dst[j], 0.2)
  s masked where ~(adj | I); softmax over j; out = p @ h_prime + bias

Sharding: data-parallel over batch - one graph per core (BS=8, 8 cores).

Hybrid per-(head, jb) tile paths, scores transposed (p[j, i]) so the
output matmul contracts over the partition axis:
  - A-path (ACT-heavy, 4 tiles/head): PE matmuls build
    s = dst_j + src_i - 144*invalid in PSUM (two k=1 outer products from
    single-row scatter tiles + an identity matmul folding the additive
    mask), then ACT Prelu + ACT Exp. exp(s - 144) ~ 0 masks invalid edges.
  - V-path (DVE-heavy, 4 tiles/head): rank-1 factorization of the
    reference nonlinearity,
       exp(leaky(x)) = max(exp(x), exp(0.2 x)) = max(E_i*F_j, E~_i*F~_j)
    with E=exp(src), F=exp(dst), E~=exp(0.2 src), F~=exp(0.2 dst): two
    DVE tensor_scalar 4x passes (replicated E rows x per-partition F
    scalars), a DVE tensor max, and a multiplicative {0,1} mask - no N*N
    exp, no PE score streams, no PSUM. Pool/GPSIMD has no ALU ops on TRN2
    (DMA/copy/memset only), so these stay on DVE; the A/V split balances
    ACT vs DVE occupancy (~84 us each).
  - src/dst rows per head pair via one [8, N] PE matmul against weight
    columns (src, dst, and 0.2-scaled copies), one ACT Exp gives all of
    E/F/E~/F~; rows go through a DRAM scratch and come back as
    partition-replicated [128, 2, N] tiles via HWDGE broadcast loads
    (stride-0 leading dim). Per-partition F columns via 8-row PE
    transposes. Raw src/dst rows are DMA-scattered into per-pair
    single-row tiles (no init needed, no cross-pair WAW serialization).
  - adj loads via casting SWDGE DMAs (u8 -> bf16 {0,1}, 8x128
    descriptors to keep the 1024-desc SWDGE ring fluid), DMA-transposed
    on the sync HWDGE queue interleaved with the pair-loop stores so the
    in-order SEQ reaches each as its data lands; diag forced valid with
    an identity max; additive A-mask derived as 144v-144.
  - bias folded into hp via an extra k=1 PE matmul (ones x bias-row);
    softmax normalization makes (sum p*(hp+bias))/sum p = out + bias
    exact; the hp ones-column yields the softmax denominator inside the
    output matmul; out stores via HWDGE on the SP queue.
  - phase B software-pipelined one head deep; emission order is tuned
    against the in-order per-engine SEQ FIFOs (products -> maxes ->
    masks batches; pair heads/tails staggered).
"""

import sys

import numpy as np

try:
    import concourse.bass  # noqa: F401
except ImportError:
    sys.path.insert(0, "/opt/trn_rl_repo")

BS, N, H, FI, FO = 8, 1024, 8, 256, 64
P = 128
NB = N // P     # 8 node blocks
FC = FI // P    # 2 f_in chunks
HP = H // 2     # head pairs
ALPHA = 0.2
BIG = 144.0

# per-head A-path jb positions (4 per head: 32 A-tiles / 32 V-tiles).
# Pool has no ALU ops on TRN2 (DMA/copy/memset only), so all V-path
# max/min ops run on DVE; the A/V split balances ACT vs DVE.
_A_COUNT = [4, 4, 4, 4, 4, 4, 4, 4]
A_JB = {
    hd: set(((hd + t) % NB) for t in range(_A_COUNT[hd]))
    for hd in range(H)
}
def _v_eng(kv):
    return "D", "D"


def build_bass():
    import concourse.bass as bass
    import concourse.mybir as mybir
    from concourse import bacc
    from concourse.masks import make_identity
    from concourse.tile import TileContext

    f32 = mybir.dt.float32
    bf16 = mybir.dt.bfloat16
    u8 = mybir.dt.uint8
    AF = mybir.ActivationFunctionType
    OP = mybir.AluOpType

    nc = bacc.Bacc(trn_type="TRN2")

    h_d = nc.dram_tensor("h", [N, FI], f32, kind="ExternalInput")
    adj_d = nc.dram_tensor("adj", [N, N], u8, kind="ExternalInput")
    w_d = nc.dram_tensor("w", [H, FI, FO], f32, kind="ExternalInput")
    asrc_d = nc.dram_tensor("a_src", [H, FO], f32, kind="ExternalInput")
    adst_d = nc.dram_tensor("a_dst", [H, FO], f32, kind="ExternalInput")
    bias_d = nc.dram_tensor("bias", [FO], f32, kind="ExternalInput")
    out_d = nc.dram_tensor("out", [H, N, FO], f32, kind="ExternalOutput")
    # E/E~/F/F~ rows staged through DRAM for partition-replication loads
    scr_d = nc.dram_tensor("scr", [8 * HP, N], bf16, kind="Internal")

    with TileContext(nc) as tc:
        with (
            tc.tile_pool(name="singles", bufs=1) as singles,
            tc.tile_pool(name="temps", bufs=2) as temps,
            tc.tile_pool(name="ppool", bufs=2) as ppool,
            tc.tile_pool(name="spool", bufs=2) as spool,
            tc.tile_pool(name="epool", bufs=4) as epool,
        ):
            # ---------- long-lived tiles ----------
            ident = singles.tile([P, P], f32)
            make_identity(nc, ident)
            identb = singles.tile([P, P], bf16)
            nc.gpsimd.tensor_copy(out=identb, in_=ident)

            hT_sb = singles.tile([P, FC, N], bf16)  # [f_local, fc, n]
            # A-path score operands: per-pair single-row tiles (scatters
            # fully overwrite them - no init, no WAW serialization). Scores
            # use two k=1 matmuls (dst x ones + ones x src) + mask fold.
            dstp, srcp = [], []
            for q in range(HP):
                dstp.append(singles.tile([1, 2, N], bf16, name=f"dstp{q}"))
                srcp.append(singles.tile([1, 2, N], bf16, name=f"srcp{q}"))
            ones_row = singles.tile([1, P], bf16)
            nc.vector.memset(ones_row, 1.0)
            ones1k = singles.tile([1, N], bf16)
            nc.vector.memset(ones1k, 1.0)

            # hp with ones column (col FO); bias folded in via PE matmul
            hp_all = singles.tile([P, NB, H, FO + 1], bf16)
            nc.vector.memset(hp_all[:, :, :, FO : FO + 1], 1.0)

            # mask forms [j_local, jb, i]: vT {0,1} multiplies V-tiles
            # directly; maskAdd = 144v-144 is the A-path additive form
            vT = singles.tile([P, NB, N], bf16)
            maskAdd = singles.tile([P, NB, N], bf16)

            # V-path replicated rows: rotating pool of [P, {E,E~}, N]
            # per-partition scalars: [j_local, jb, pair, row8] (f32)
            cols = singles.tile([P, NB, HP, 8], f32)
            erep_t = {}

            # ================= phase A: prep =================
            with (
                tc.tile_pool(name="phA", bufs=1) as phA,
                tc.tile_pool(name="tempA", bufs=4) as tempA,
                tc.tile_pool(name="psumA", bufs=2, space="PSUM") as psA,
                tc.tile_pool(name="psumHP", bufs=1, space="PSUM") as psHP,
                tc.tile_pool(name="psumR", bufs=2, space="PSUM") as psR,
                tc.tile_pool(name="psumD", bufs=1, space="PSUM") as psD,
                tc.tile_pool(name="psumC", bufs=1, space="PSUM") as psC,
            ):
                # w and h first (HWDGE f32 loads - no SWDGE ring space,
                # hardware desc-gen) + on-chip bf16 converts; they gate the
                # tanh/psd chain everything else hangs off
                w_f = phA.tile([P, FC, H, FO], f32)
                w_re = w_d.rearrange("h (c p) o -> p c h o", p=P)
                w_sb = phA.tile([P, FC, H, FO], bf16)
                h_sb = phA.tile([P, NB, FI], f32)
                h_re = h_d.rearrange("(nb p) f -> p nb f", p=P)
                nc.sync.dma_start(out=w_f[:, 0], in_=w_re[:, 0])
                nc.sync.dma_start(out=h_sb[:, 0:4], in_=h_re[:, 0:4])
                nc.vector.tensor_copy(out=w_sb[:, 0], in_=w_f[:, 0])
                nc.sync.dma_start(out=w_f[:, 1], in_=w_re[:, 1])
                nc.sync.dma_start(out=h_sb[:, 4:8], in_=h_re[:, 4:8])
                nc.vector.tensor_copy(out=w_sb[:, 1], in_=w_f[:, 1])
                # adjacency: casting SWDGE DMA u8 -> bf16 {0.0, 1.0};
                # DMA-transposes follow immediately on the scalar HWDGE
                # queue (keeps the sync queue free for scr/erep/out traffic)
                adjb = phA.tile([P, NB, N], bf16)
                adj_re = adj_d.rearrange("(ib p) j -> p ib j", p=P)
                for ib in range(NB):
                    nc.gpsimd.dma_start(
                        out=adjb[:, ib : ib + 1],
                        in_=adj_re[:, ib : ib + 1],
                    )
                for ib in range(NB):
                    nc.sync.dma_start_transpose(
                        out=vT[:, :, ib * P : (ib + 1) * P],
                        in_=adjb[:, ib],
                    )

                # a_src/a_dst: load as [16, 64], transpose to [64, 16]
                a2d = phA.tile([2 * H, FO], f32)
                nc.sync.dma_start(out=a2d[0:H], in_=asrc_d[:, :])
                nc.sync.dma_start(out=a2d[H : 2 * H], in_=adst_d[:, :])

                # bias row replicated over heads, bf16, with 0 in col FO
                bias_f = phA.tile([1, FO], f32)
                nc.sync.dma_start(
                    out=bias_f,
                    in_=bass.AP(tensor=bias_d, offset=0, ap=[[0, 1], [1, FO]]),
                )
                bias_hrow = phA.tile([1, H, FO + 1], bf16)
                nc.vector.memset(bias_hrow, 0.0)
                nc.vector.tensor_copy(
                    out=bias_hrow[:, :, 0:FO],
                    in_=bass.AP(
                        tensor=bias_f.tensor,
                        offset=bias_f.offset,
                        ap=[list(bias_f.ap[0]), [0, H], [1, FO]],
                    ),
                )

                # a2T: [64, 16] weight columns; a2p8 adds 0.2-scaled copies
                pa2 = psR.tile([FO, 2 * H], f32, tag="hpT")
                nc.tensor.transpose(pa2, a2d, ident[0 : 2 * H, 0 : 2 * H])
                a2T = phA.tile([FO, 2 * H], bf16)
                nc.vector.tensor_copy(out=a2T, in_=pa2)
                # paired-head weight cols: a2p8[:, m, q]; head 2q on parts
                # 0-63, head 2q+1 on 64-127; m = (src, dst, src', dst',
                # .2src, .2dst, .2src', .2dst')
                a2p8 = phA.tile([P, 8, HP], bf16)
                nc.vector.memset(a2p8, 0.0)
                for q in range(HP):
                    nc.vector.tensor_copy(
                        out=a2p8[0:FO, 0:2, q],
                        in_=bass.AP(
                            tensor=a2T.tensor, offset=a2T.offset + 2 * q,
                            ap=[list(a2T.ap[0]), [H, 2]],
                        ),
                    )
                    nc.vector.tensor_copy(
                        out=a2p8[FO : 2 * FO, 2:4, q],
                        in_=bass.AP(
                            tensor=a2T.tensor, offset=a2T.offset + 2 * q + 1,
                            ap=[list(a2T.ap[0]), [H, 2]],
                        ),
                    )
                nc.vector.tensor_scalar(
                    out=a2p8[:, 4:8, :], in0=a2p8[:, 0:4, :],
                    scalar1=ALPHA, scalar2=None, op0=OP.mult,
                )

                # hT: h transposed to [f_local, fc, n]
                for nb in range(NB):
                    for fc in range(FC):
                        pt = psA.tile([P, P], f32, tag="tr")
                        nc.tensor.transpose(
                            pt, h_sb[:, nb, fc * P : (fc + 1) * P], ident
                        )
                        nc.scalar.activation(
                            out=hT_sb[:, fc, nb * P : (nb + 1) * P],
                            in_=pt, func=AF.Copy,
                        )

                # ----- src/dst rows via paired transposed h_prime -----
                # psd8 rows: (src2q, dst2q, src2q+1, dst2q+1) then 0.2x
                # copies; one ACT Exp per pair gives (E,F,E',F',E~,...)
                ptc = psC.tile([P, NB, HP, 8], bf16, tag="cols")
                st8s = {}

                def emit_diag(ib):
                    nc.vector.tensor_max(
                        out=vT[:, ib, ib * P : (ib + 1) * P],
                        in0=vT[:, ib, ib * P : (ib + 1) * P],
                        in1=identb,
                    )

                def emit_derives(half):
                    hs = slice(half * 512, (half + 1) * 512)
                    for jb in range(NB):
                        nc.vector.tensor_scalar(
                            out=maskAdd[:, jb, hs], in0=vT[:, jb, hs],
                            scalar1=BIG, scalar2=-BIG,
                            op0=OP.mult, op1=OP.add,
                        )

                def emit_pair_head(q):
                    # tanh chain -> psd8 -> SBUF st8; the DVE copy alone
                    # frees the PSUM tile so pairs advance at tanh rate
                    tTp = tempA.tile([P, N], bf16, tag="tT")
                    st8 = phA.tile([8, N], bf16, tag=f"st8{q}")
                    for half in range(2):
                        hs = slice(half * 512, (half + 1) * 512)
                        phT = psR.tile([P, 512], f32, tag="hpT")
                        for fc in range(FC):
                            nc.tensor.matmul(
                                phT,
                                lhsT=w_sb[:, fc, 2 * q : 2 * q + 2, :],
                                rhs=hT_sb[:, fc, hs],
                                start=(fc == 0),
                                stop=(fc == FC - 1),
                                skip_group_check=True,
                            )
                        nc.scalar.activation(
                            out=tTp[:, hs], in_=phT, func=AF.Tanh
                        )
                        psd8 = psD.tile([8, 512], f32, tag="sd")
                        nc.tensor.matmul(
                            psd8, lhsT=a2p8[:, :, q], rhs=tTp[:, hs],
                            start=True, stop=True,
                        )
                        nc.scalar.activation(
                            out=st8[:, hs], in_=psd8, func=AF.Copy
                        )
                    st8s[q] = st8

                def emit_pair_tail(q):
                    # rows: (src2q, dst2q, src2q+1, dst2q+1, 0.2x those)
                    st8 = st8s[q]
                    eAll = phA.tile([8, N], bf16, tag=f"eAll{q}")
                    nc.scalar.activation(out=eAll, in_=st8, func=AF.Exp)
                    # to DRAM scratch for broadcast-loads
                    nc.sync.dma_start(out=scr_d[8 * q : 8 * q + 8, :], in_=eAll)
                    # per-partition scalar columns via PE transposes
                    for jb in range(NB):
                        nc.tensor.transpose(
                            ptc[:, jb, q, :],
                            eAll[:, jb * P : (jb + 1) * P],
                            identb[0:8, 0:8],
                        )
                    nc.vector.tensor_copy(
                        out=cols[:, :, q, :], in_=ptc[:, :, q, :]
                    )
                    # broadcast-load E/E~ replicated rows for both heads
                    for r in range(2):
                        hd = 2 * q + r
                        et = epool.tile([P, 2, N], bf16, tag="erep")
                        nc.sync.dma_start(
                            out=et,
                            in_=bass.AP(
                                tensor=scr_d,
                                offset=(8 * q + 2 * r) * N,
                                ap=[[0, P], [4 * N, 2], [1, N]],
                            ),
                        )
                        erep_t[hd] = et
                    # scatter raw rows into A-path operands:
                    # dsts (st8 rows 1,3) -> dstp[q], srcs (0,2) -> srcp[q]
                    nc.gpsimd.dma_start(
                        out=dstp[q][0:1, :, :],
                        in_=bass.AP(
                            tensor=st8.tensor, offset=st8.offset + N,
                            ap=[[2 * N, 2], [1, N]],
                        ),
                    )
                    nc.gpsimd.dma_start(
                        out=srcp[q][0:1, :, :],
                        in_=bass.AP(
                            tensor=st8.tensor, offset=st8.offset,
                            ap=[[2 * N, 2], [1, N]],
                        ),
                    )
                for q in range(HP):
                    emit_pair_head(q)
                    if q >= 1:
                        emit_pair_tail(q - 1)
                emit_pair_tail(HP - 1)
                # adj diag forcing + additive-mask derivation AFTER the pair
                # loop: keeps the DVE FIFO clear of transpose-gated work
                # while the pair chain drains
                for ib in range(NB):
                    emit_diag(ib)
                for half in range(2):
                    emit_derives(half)

                # ----- hp (+ones col, bias via extra matmul) -----
                for nb in range(NB):
                    php = psHP.tile([P, H, FO], f32, tag="hp")
                    nc.tensor.matmul(
                        php,
                        lhsT=ones_row,
                        rhs=bias_hrow[:, :, 0:FO],
                        start=True, stop=False,
                    )
                    for fc in range(FC):
                        nc.tensor.matmul(
                            php,
                            lhsT=hT_sb[:, fc, nb * P : (nb + 1) * P],
                            rhs=w_sb[:, fc],
                            start=False,
                            stop=(fc == FC - 1),
                        )
                    nc.vector.tensor_copy(out=hp_all[:, nb, :, 0:FO], in_=php)

            # ================= phase B: attention =================
            with (
                tc.tile_pool(name="psumS", bufs=3, space="PSUM") as psS,
                tc.tile_pool(name="psumO", bufs=1, space="PSUM") as psO,
            ):
                def emit_scores(hd):
                    q, r = hd // 2, hd % 2
                    p_sb = ppool.tile([P, NB, N], bf16, tag="p")
                    a_jbs = [jb for jb in range(NB) if jb in A_JB[hd]]
                    v_jbs = [jb for jb in range(NB) if jb not in A_JB[hd]]
                    # A-path first: PE matmuls + ACT prelu/exp
                    for jb in a_jbs:
                        ps = psS.tile([P, N], f32, tag="spre")
                        for half in range(2):
                            hs = slice(half * 512, (half + 1) * 512)
                            nc.tensor.matmul(
                                ps[:, hs],
                                lhsT=dstp[q][:, r, jb * P : (jb + 1) * P],
                                rhs=ones1k[:, hs],
                                start=True,
                                stop=False,
                                skip_group_check=True,
                            )
                            nc.tensor.matmul(
                                ps[:, hs],
                                lhsT=ones_row,
                                rhs=srcp[q][:, r, hs],
                                start=False,
                                stop=False,
                                skip_group_check=True,
                            )
                            nc.tensor.matmul(
                                ps[:, hs],
                                lhsT=identb,
                                rhs=maskAdd[:, jb, hs],
                                start=False,
                                stop=True,
                                skip_group_check=True,
                            )
                        nc.scalar.activation(
                            out=ps, in_=ps, func=AF.Prelu, alpha=ALPHA
                        )
                        nc.scalar.activation(
                            out=p_sb[:, jb, :], in_=ps, func=AF.Exp
                        )
                    # V-path: g = E_rep*F_j, g~ = E~_rep*F~_j, max, min-mask
                    # batched: all products, then maxes, then masks, so the
                    # in-order DVE queue never head-of-line blocks on Pool
                    gs, gts, us = {}, {}, {}
                    for jb in v_jbs:
                        g = spool.tile([P, N], bf16, tag="g")
                        nc.vector.tensor_scalar(
                            out=g, in0=erep_t[hd][:, 0, :],
                            scalar1=cols[:, jb, q, 1 + 2 * r : 2 + 2 * r],
                            scalar2=None, op0=OP.mult,
                        )
                        gt = spool.tile([P, N], bf16, tag="gt")
                        nc.vector.tensor_scalar(
                            out=gt, in0=erep_t[hd][:, 1, :],
                            scalar1=cols[:, jb, q, 5 + 2 * r : 6 + 2 * r],
                            scalar2=None, op0=OP.mult,
                        )
                        gs[jb], gts[jb] = g, gt
                    for jb in v_jbs:
                        u = spool.tile([P, N], bf16, tag="u")
                        nc.vector.tensor_max(out=u, in0=gs[jb], in1=gts[jb])
                        us[jb] = u
                    for jb in v_jbs:
                        nc.vector.tensor_mul(
                            out=p_sb[:, jb, :], in0=us[jb],
                            in1=vT[:, jb, :],
                        )
                    return p_sb

                def emit_out(hd, p_sb):
                    out_re = out_d[hd].rearrange("(p ic) o -> p ic o", ic=NB)
                    rz = temps.tile([P, NB, 1], f32, tag="rz")
                    o_sb = temps.tile([P, NB, FO], f32, tag="osb")
                    for grp in range(2):
                        po = psO.tile([P, 4, FO + 1], f32,
                                      tag="o2a" if grp == 0 else "o2b")
                        ics = range(4 * grp, 4 * grp + 4)
                        for ic in ics:
                            icl = ic % 4
                            for jb in range(NB):
                                lhsT_str = bass.AP(
                                    tensor=p_sb.tensor,
                                    offset=p_sb[:, jb, ic : ic + 1].offset,
                                    ap=[list(p_sb.ap[0]), [NB, P]],
                                )
                                nc.tensor.matmul(
                                    po[:, icl, :],
                                    lhsT=lhsT_str,
                                    rhs=hp_all[:, jb, hd, :],
                                    start=(jb == 0),
                                    stop=(jb == NB - 1),
                                )
                        gs = slice(4 * grp, 4 * grp + 4)
                        nc.vector.reciprocal(
                            out=rz[:, gs], in_=po[:, :, FO : FO + 1]
                        )
                        rzb = bass.AP(
                            tensor=rz.tensor, offset=rz[:, gs, :].offset,
                            ap=[list(rz.ap[0]), [rz.ap[1][0], 4], [0, FO]],
                        )
                        nc.vector.tensor_mul(
                            out=o_sb[:, gs, :], in0=po[:, :, 0:FO], in1=rzb
                        )
                        nc.sync.dma_start(
                            out=out_re[:, gs], in_=o_sb[:, gs]
                        )

                prev_p = None
                for hd in range(H):
                    p_new = emit_scores(hd)
                    if prev_p is not None:
                        emit_out(hd - 1, prev_p)
                    prev_p = p_new
                emit_out(H - 1, prev_p)
    nc.finalize()
    return nc


_NC_CACHE = None
TRACE = False
LAST_RESULT = None


def kernel(h, adj, w, a_src, a_dst, bias):
    global _NC_CACHE
    from concourse.bass_utils import run_bass_kernel_spmd

    if _NC_CACHE is None:
        _NC_CACHE = build_bass()
    nc = _NC_CACHE

    h = np.ascontiguousarray(np.asarray(h, dtype=np.float32))
    adj_u8 = np.ascontiguousarray(np.asarray(adj).astype(np.uint8))
    w = np.ascontiguousarray(np.asarray(w, dtype=np.float32))
    a_src2 = np.ascontiguousarray(np.asarray(a_src, dtype=np.float32)[..., 0])
    a_dst2 = np.ascontiguousarray(np.asarray(a_dst, dtype=np.float32)[..., 0])
    bias = np.ascontiguousarray(np.asarray(bias, dtype=np.float32))

    in_maps = [
        {
            "h": h[b],
            "adj": adj_u8[b],
            "w": w,
            "a_src": a_src2,
            "a_dst": a_dst2,
            "bias": bias,
        }
        for b in range(BS)
    ]
    res = run_bass_kernel_spmd(
        nc, in_maps, core_ids=list(range(BS)), trace=TRACE,
        trace_cores=list(range(BS)) if TRACE else None,
    )
    if TRACE:
        global LAST_RESULT
        LAST_RESULT = res
    out = np.stack([r["out"] for r in res.results], axis=0)
    return out.astype(np.float32)


# revision 63
# speedup vs baseline: 1.0209x; 1.0075x over previous
"""BatchMultiHeadGraphAttention kernel for TRN2 (8 NeuronCores).

Reference computation (per graph b):
  h_prime = h @ w[head]                 [n, fo] per head
  t = tanh(h_prime)
  src[n] = t @ a_src[head];  dst[n] = t @ a_dst[head]
  s[i, j] = leaky_relu(src[i] + dst[j], 0.2)
  s masked where ~(adj | I); softmax over j; out = p @ h_prime + bias

Sharding: data-parallel over batch - one graph per core (BS=8, 8 cores).

Hybrid per-(head, jb) tile paths, scores transposed (p[j, i]) so the
output matmul contracts over the partition axis:
  - A-path (ACT-heavy, 4 tiles/head): PE matmuls build
    s = dst_j + src_i - 144*invalid in PSUM (two k=1 outer products from
    single-row scatter tiles + an identity matmul folding the additive
    mask), then ACT Prelu + ACT Exp. exp(s - 144) ~ 0 masks invalid edges.
  - V-path (DVE-heavy, 4 tiles/head): rank-1 factorization of the
    reference nonlinearity,
       exp(leaky(x)) = max(exp(x), exp(0.2 x)) = max(E_i*F_j, E~_i*F~_j)
    with E=exp(src), F=exp(dst), E~=exp(0.2 src), F~=exp(0.2 dst): two
    DVE tensor_scalar 4x passes (replicated E rows x per-partition F
    scalars), a DVE tensor max, and a multiplicative {0,1} mask - no N*N
    exp, no PE score streams, no PSUM. Pool/GPSIMD has no ALU ops on TRN2
    (DMA/copy/memset only), so these stay on DVE; the A/V split balances
    ACT vs DVE occupancy (~84 us each).
  - src/dst rows per head pair via one [8, N] PE matmul against weight
    columns (src, dst, and 0.2-scaled copies), one ACT Exp gives all of
    E/F/E~/F~; rows go through a DRAM scratch and come back as
    partition-replicated [128, 2, N] tiles via HWDGE broadcast loads
    (stride-0 leading dim). Per-partition F columns via 8-row PE
    transposes. Raw src/dst rows are DMA-scattered into per-pair
    single-row tiles (no init needed, no cross-pair WAW serialization).
  - adj loads via casting SWDGE DMAs (u8 -> bf16 {0,1}, 8x128
    descriptors to keep the 1024-desc SWDGE ring fluid), DMA-transposed
    on the sync HWDGE queue interleaved with the pair-loop stores so the
    in-order SEQ reaches each as its data lands; diag forced valid with
    an identity max; additive A-mask derived as 144v-144.
  - bias folded into hp via an extra k=1 PE matmul (ones x bias-row);
    softmax normalization makes (sum p*(hp+bias))/sum p = out + bias
    exact; the hp ones-column yields the softmax denominator inside the
    output matmul; out stores via HWDGE on the SP queue.
  - phase B software-pipelined one head deep; emission order is tuned
    against the in-order per-engine SEQ FIFOs (products -> maxes ->
    masks batches; pair heads/tails staggered).
"""

import sys

import numpy as np

try:
    import concourse.bass  # noqa: F401
except ImportError:
    sys.path.insert(0, "/opt/trn_rl_repo")

BS, N, H, FI, FO = 8, 1024, 8, 256, 64
P = 128
NB = N // P     # 8 node blocks
FC = FI // P    # 2 f_in chunks
HP = H // 2     # head pairs
ALPHA = 0.2
BIG = 144.0

# per-head A-path jb positions (4 per head: 32 A-tiles / 32 V-tiles).
# Pool has no ALU ops on TRN2 (DMA/copy/memset only), so all V-path
# max/min ops run on DVE; the A/V split balances ACT vs DVE.
_A_COUNT = [4, 4, 4, 4, 4, 4, 4, 4]
A_JB = {
    hd: set(((hd + t) % NB) for t in range(_A_COUNT[hd]))
    for hd in range(H)
}
def _v_eng(kv):
    return "D", "D"


def build_bass():
    import concourse.bass as bass
    import concourse.mybir as mybir
    from concourse import bacc
    from concourse.masks import make_identity
    from concourse.tile import TileContext

    f32 = mybir.dt.float32
    bf16 = mybir.dt.bfloat16
    u8 = mybir.dt.uint8
    AF = mybir.ActivationFunctionType
    OP = mybir.AluOpType

    nc = bacc.Bacc(trn_type="TRN2")

    h_d = nc.dram_tensor("h", [N, FI], f32, kind="ExternalInput")
    adj_d = nc.dram_tensor("adj", [N, N], u8, kind="ExternalInput")
    w_d = nc.dram_tensor("w", [H, FI, FO], f32, kind="ExternalInput")
    asrc_d = nc.dram_tensor("a_src", [H, FO], f32, kind="ExternalInput")
    adst_d = nc.dram_tensor("a_dst", [H, FO], f32, kind="ExternalInput")
    bias_d = nc.dram_tensor("bias", [FO], f32, kind="ExternalInput")
    out_d = nc.dram_tensor("out", [H, N, FO], f32, kind="ExternalOutput")
    # E/E~/F/F~ rows staged through DRAM for partition-replication loads
    scr_d = nc.dram_tensor("scr", [8 * HP, N], bf16, kind="Internal")

    with TileContext(nc) as tc:
        with (
            tc.tile_pool(name="singles", bufs=1) as singles,
            tc.tile_pool(name="temps", bufs=2) as temps,
            tc.tile_pool(name="ppool", bufs=2) as ppool,
            tc.tile_pool(name="spool", bufs=3) as spool,
            tc.tile_pool(name="epool", bufs=4) as epool,
        ):
            # ---------- long-lived tiles ----------
            ident = singles.tile([P, P], f32)
            make_identity(nc, ident)
            identb = singles.tile([P, P], bf16)
            nc.gpsimd.tensor_copy(out=identb, in_=ident)

            hT_sb = singles.tile([P, FC, N], bf16)  # [f_local, fc, n]
            # A-path score operands: per-pair single-row tiles (scatters
            # fully overwrite them - no init, no WAW serialization). Scores
            # use two k=1 matmuls (dst x ones + ones x src) + mask fold.
            dstp, srcp = [], []
            for q in range(HP):
                dstp.append(singles.tile([1, 2, N], bf16, name=f"dstp{q}"))
                srcp.append(singles.tile([1, 2, N], bf16, name=f"srcp{q}"))
            ones_row = singles.tile([1, P], bf16)
            nc.vector.memset(ones_row, 1.0)
            ones1k = singles.tile([1, N], bf16)
            nc.vector.memset(ones1k, 1.0)

            # hp with ones column (col FO); bias folded in via PE matmul
            hp_all = singles.tile([P, NB, H, FO + 1], bf16)
            nc.vector.memset(hp_all[:, :, :, FO : FO + 1], 1.0)

            # mask forms [j_local, jb, i]: vT {0,1} multiplies V-tiles
            # directly; maskAdd = 144v-144 is the A-path additive form
            vT = singles.tile([P, NB, N], bf16)
            maskAdd = singles.tile([P, NB, N], bf16)

            # V-path replicated rows: rotating pool of [P, {E,E~}, N]
            # per-partition scalars: [j_local, jb, pair, row8] (f32)
            cols = singles.tile([P, NB, HP, 8], f32)
            erep_t = {}

            # ================= phase A: prep =================
            with (
                tc.tile_pool(name="phA", bufs=1) as phA,
                tc.tile_pool(name="tempA", bufs=4) as tempA,
                tc.tile_pool(name="psumA", bufs=2, space="PSUM") as psA,
                tc.tile_pool(name="psumHP", bufs=1, space="PSUM") as psHP,
                tc.tile_pool(name="psumR", bufs=2, space="PSUM") as psR,
                tc.tile_pool(name="psumD", bufs=1, space="PSUM") as psD,
                tc.tile_pool(name="psumC", bufs=1, space="PSUM") as psC,
            ):
                # w and h first (HWDGE f32 loads - no SWDGE ring space,
                # hardware desc-gen) + on-chip bf16 converts; they gate the
                # tanh/psd chain everything else hangs off
                w_f = phA.tile([P, FC, H, FO], f32)
                w_re = w_d.rearrange("h (c p) o -> p c h o", p=P)
                w_sb = phA.tile([P, FC, H, FO], bf16)
                h_sb = phA.tile([P, NB, FI], f32)
                h_re = h_d.rearrange("(nb p) f -> p nb f", p=P)
                nc.sync.dma_start(out=w_f[:, 0], in_=w_re[:, 0])
                nc.sync.dma_start(out=h_sb[:, 0:4], in_=h_re[:, 0:4])
                nc.vector.tensor_copy(out=w_sb[:, 0], in_=w_f[:, 0])
                nc.sync.dma_start(out=w_f[:, 1], in_=w_re[:, 1])
                nc.sync.dma_start(out=h_sb[:, 4:8], in_=h_re[:, 4:8])
                nc.vector.tensor_copy(out=w_sb[:, 1], in_=w_f[:, 1])
                # adjacency: casting SWDGE DMA u8 -> bf16 {0.0, 1.0};
                # DMA-transposes follow immediately on the scalar HWDGE
                # queue (keeps the sync queue free for scr/erep/out traffic)
                adjb = phA.tile([P, NB, N], bf16)
                adj_re = adj_d.rearrange("(ib p) j -> p ib j", p=P)
                for ib in range(NB):
                    nc.gpsimd.dma_start(
                        out=adjb[:, ib : ib + 1],
                        in_=adj_re[:, ib : ib + 1],
                    )
                for ib in range(NB):
                    nc.sync.dma_start_transpose(
                        out=vT[:, :, ib * P : (ib + 1) * P],
                        in_=adjb[:, ib],
                    )

                # a_src/a_dst: load as [16, 64], transpose to [64, 16]
                a2d = phA.tile([2 * H, FO], f32)
                nc.sync.dma_start(out=a2d[0:H], in_=asrc_d[:, :])
                nc.sync.dma_start(out=a2d[H : 2 * H], in_=adst_d[:, :])

                # bias row replicated over heads, bf16, with 0 in col FO
                bias_f = phA.tile([1, FO], f32)
                nc.sync.dma_start(
                    out=bias_f,
                    in_=bass.AP(tensor=bias_d, offset=0, ap=[[0, 1], [1, FO]]),
                )
                bias_hrow = phA.tile([1, H, FO + 1], bf16)
                nc.vector.memset(bias_hrow, 0.0)
                nc.vector.tensor_copy(
                    out=bias_hrow[:, :, 0:FO],
                    in_=bass.AP(
                        tensor=bias_f.tensor,
                        offset=bias_f.offset,
                        ap=[list(bias_f.ap[0]), [0, H], [1, FO]],
                    ),
                )

                # a2T: [64, 16] weight columns; a2p8 adds 0.2-scaled copies
                pa2 = psR.tile([FO, 2 * H], f32, tag="hpT")
                nc.tensor.transpose(pa2, a2d, ident[0 : 2 * H, 0 : 2 * H])
                a2T = phA.tile([FO, 2 * H], bf16)
                nc.vector.tensor_copy(out=a2T, in_=pa2)
                # paired-head weight cols: a2p8[:, m, q]; head 2q on parts
                # 0-63, head 2q+1 on 64-127; m = (src, dst, src', dst',
                # .2src, .2dst, .2src', .2dst')
                a2p8 = phA.tile([P, 8, HP], bf16)
                nc.vector.memset(a2p8, 0.0)
                for q in range(HP):
                    nc.vector.tensor_copy(
                        out=a2p8[0:FO, 0:2, q],
                        in_=bass.AP(
                            tensor=a2T.tensor, offset=a2T.offset + 2 * q,
                            ap=[list(a2T.ap[0]), [H, 2]],
                        ),
                    )
                    nc.vector.tensor_copy(
                        out=a2p8[FO : 2 * FO, 2:4, q],
                        in_=bass.AP(
                            tensor=a2T.tensor, offset=a2T.offset + 2 * q + 1,
                            ap=[list(a2T.ap[0]), [H, 2]],
                        ),
                    )
                nc.vector.tensor_scalar(
                    out=a2p8[:, 4:8, :], in0=a2p8[:, 0:4, :],
                    scalar1=ALPHA, scalar2=None, op0=OP.mult,
                )

                # hT: h transposed to [f_local, fc, n]
                for nb in range(NB):
                    for fc in range(FC):
                        pt = psA.tile([P, P], f32, tag="tr")
                        nc.tensor.transpose(
                            pt, h_sb[:, nb, fc * P : (fc + 1) * P], ident
                        )
                        nc.scalar.activation(
                            out=hT_sb[:, fc, nb * P : (nb + 1) * P],
                            in_=pt, func=AF.Copy,
                        )

                # ----- src/dst rows via paired transposed h_prime -----
                # psd8 rows: (src2q, dst2q, src2q+1, dst2q+1) then 0.2x
                # copies; one ACT Exp per pair gives (E,F,E',F',E~,...)
                ptc = psC.tile([P, NB, HP, 8], bf16, tag="cols")
                st8s = {}

                def emit_diag(ib):
                    nc.vector.tensor_max(
                        out=vT[:, ib, ib * P : (ib + 1) * P],
                        in0=vT[:, ib, ib * P : (ib + 1) * P],
                        in1=identb,
                    )

                def emit_derives(half):
                    hs = slice(half * 512, (half + 1) * 512)
                    for jb in range(NB):
                        nc.vector.tensor_scalar(
                            out=maskAdd[:, jb, hs], in0=vT[:, jb, hs],
                            scalar1=BIG, scalar2=-BIG,
                            op0=OP.mult, op1=OP.add,
                        )

                def emit_pair_head(q):
                    # tanh chain -> psd8 -> SBUF st8; the DVE copy alone
                    # frees the PSUM tile so pairs advance at tanh rate
                    tTp = tempA.tile([P, N], bf16, tag="tT")
                    st8 = phA.tile([8, N], bf16, tag=f"st8{q}")
                    for half in range(2):
                        hs = slice(half * 512, (half + 1) * 512)
                        phT = psR.tile([P, 512], f32, tag="hpT")
                        for fc in range(FC):
                            nc.tensor.matmul(
                                phT,
                                lhsT=w_sb[:, fc, 2 * q : 2 * q + 2, :],
                                rhs=hT_sb[:, fc, hs],
                                start=(fc == 0),
                                stop=(fc == FC - 1),
                                skip_group_check=True,
                            )
                        nc.scalar.activation(
                            out=tTp[:, hs], in_=phT, func=AF.Tanh
                        )
                        psd8 = psD.tile([8, 512], f32, tag="sd")
                        nc.tensor.matmul(
                            psd8, lhsT=a2p8[:, :, q], rhs=tTp[:, hs],
                            start=True, stop=True,
                        )
                        nc.scalar.activation(
                            out=st8[:, hs], in_=psd8, func=AF.Copy
                        )
                    st8s[q] = st8

                def emit_pair_tail(q):
                    # rows: (src2q, dst2q, src2q+1, dst2q+1, 0.2x those)
                    st8 = st8s[q]
                    eAll = phA.tile([8, N], bf16, tag=f"eAll{q}")
                    nc.scalar.activation(out=eAll, in_=st8, func=AF.Exp)
                    # to DRAM scratch for broadcast-loads
                    nc.sync.dma_start(out=scr_d[8 * q : 8 * q + 8, :], in_=eAll)
                    # per-partition scalar columns via PE transposes
                    for jb in range(NB):
                        nc.tensor.transpose(
                            ptc[:, jb, q, :],
                            eAll[:, jb * P : (jb + 1) * P],
                            identb[0:8, 0:8],
                        )
                    nc.vector.tensor_copy(
                        out=cols[:, :, q, :], in_=ptc[:, :, q, :]
                    )
                    # broadcast-load E/E~ replicated rows for both heads
                    for r in range(2):
                        hd = 2 * q + r
                        et = epool.tile([P, 2, N], bf16, tag="erep")
                        nc.sync.dma_start(
                            out=et,
                            in_=bass.AP(
                                tensor=scr_d,
                                offset=(8 * q + 2 * r) * N,
                                ap=[[0, P], [4 * N, 2], [1, N]],
                            ),
                        )
                        erep_t[hd] = et
                    # scatter raw rows into A-path operands:
                    # dsts (st8 rows 1,3) -> dstp[q], srcs (0,2) -> srcp[q]
                    nc.gpsimd.dma_start(
                        out=dstp[q][0:1, :, :],
                        in_=bass.AP(
                            tensor=st8.tensor, offset=st8.offset + N,
                            ap=[[2 * N, 2], [1, N]],
                        ),
                    )
                    nc.gpsimd.dma_start(
                        out=srcp[q][0:1, :, :],
                        in_=bass.AP(
                            tensor=st8.tensor, offset=st8.offset,
                            ap=[[2 * N, 2], [1, N]],
                        ),
                    )
                for q in range(HP):
                    emit_pair_head(q)
                    if q >= 1:
                        emit_pair_tail(q - 1)
                emit_pair_tail(HP - 1)
                # adj diag forcing + additive-mask derivation AFTER the pair
                # loop: keeps the DVE FIFO clear of transpose-gated work
                # while the pair chain drains
                for ib in range(NB):
                    emit_diag(ib)
                for half in range(2):
                    emit_derives(half)

                # ----- hp (+ones col, bias via extra matmul) -----
                for nb in range(NB):
                    php = psHP.tile([P, H, FO], f32, tag="hp")
                    nc.tensor.matmul(
                        php,
                        lhsT=ones_row,
                        rhs=bias_hrow[:, :, 0:FO],
                        start=True, stop=False,
                    )
                    for fc in range(FC):
                        nc.tensor.matmul(
                            php,
                            lhsT=hT_sb[:, fc, nb * P : (nb + 1) * P],
                            rhs=w_sb[:, fc],
                            start=False,
                            stop=(fc == FC - 1),
                        )
                    nc.vector.tensor_copy(out=hp_all[:, nb, :, 0:FO], in_=php)

            # ================= phase B: attention =================
            with (
                tc.tile_pool(name="psumS", bufs=3, space="PSUM") as psS,
                tc.tile_pool(name="psumO", bufs=1, space="PSUM") as psO,
            ):
                def emit_scores(hd):
                    q, r = hd // 2, hd % 2
                    p_sb = ppool.tile([P, NB, N], bf16, tag="p")
                    a_jbs = [jb for jb in range(NB) if jb in A_JB[hd]]
                    v_jbs = [jb for jb in range(NB) if jb not in A_JB[hd]]
                    # A-path first: PE matmuls + ACT prelu/exp
                    for jb in a_jbs:
                        ps = psS.tile([P, N], f32, tag="spre")
                        for half in range(2):
                            hs = slice(half * 512, (half + 1) * 512)
                            nc.tensor.matmul(
                                ps[:, hs],
                                lhsT=dstp[q][:, r, jb * P : (jb + 1) * P],
                                rhs=ones1k[:, hs],
                                start=True,
                                stop=False,
                                skip_group_check=True,
                            )
                            nc.tensor.matmul(
                                ps[:, hs],
                                lhsT=ones_row,
                                rhs=srcp[q][:, r, hs],
                                start=False,
                                stop=False,
                                skip_group_check=True,
                            )
                            nc.tensor.matmul(
                                ps[:, hs],
                                lhsT=identb,
                                rhs=maskAdd[:, jb, hs],
                                start=False,
                                stop=True,
                                skip_group_check=True,
                            )
                        nc.scalar.activation(
                            out=ps, in_=ps, func=AF.Prelu, alpha=ALPHA
                        )
                        nc.scalar.activation(
                            out=p_sb[:, jb, :], in_=ps, func=AF.Exp
                        )
                    # V-path: g = E_rep*F_j, g~ = E~_rep*F~_j, max, min-mask
                    # batched: all products, then maxes, then masks, so the
                    # in-order DVE queue never head-of-line blocks on Pool
                    gs, gts, us = {}, {}, {}
                    for jb in v_jbs:
                        g = spool.tile([P, N], bf16, tag="g")
                        nc.vector.tensor_scalar(
                            out=g, in0=erep_t[hd][:, 0, :],
                            scalar1=cols[:, jb, q, 1 + 2 * r : 2 + 2 * r],
                            scalar2=None, op0=OP.mult,
                        )
                        gt = spool.tile([P, N], bf16, tag="gt")
                        nc.vector.tensor_scalar(
                            out=gt, in0=erep_t[hd][:, 1, :],
                            scalar1=cols[:, jb, q, 5 + 2 * r : 6 + 2 * r],
                            scalar2=None, op0=OP.mult,
                        )
                        gs[jb], gts[jb] = g, gt
                    for jb in v_jbs:
                        nc.vector.tensor_max(
                            out=gs[jb], in0=gs[jb], in1=gts[jb]
                        )
                    for jb in v_jbs:
                        nc.vector.tensor_mul(
                            out=p_sb[:, jb, :], in0=gs[jb],
                            in1=vT[:, jb, :],
                        )
                    return p_sb

                def emit_out(hd, p_sb):
                    out_re = out_d[hd].rearrange("(p ic) o -> p ic o", ic=NB)
                    rz = temps.tile([P, NB, 1], f32, tag="rz")
                    o_sb = temps.tile([P, NB, FO], f32, tag="osb")
                    for grp in range(2):
                        po = psO.tile([P, 4, FO + 1], f32,
                                      tag="o2a" if grp == 0 else "o2b")
                        ics = range(4 * grp, 4 * grp + 4)
                        for ic in ics:
                            icl = ic % 4
                            for jb in range(NB):
                                lhsT_str = bass.AP(
                                    tensor=p_sb.tensor,
                                    offset=p_sb[:, jb, ic : ic + 1].offset,
                                    ap=[list(p_sb.ap[0]), [NB, P]],
                                )
                                nc.tensor.matmul(
                                    po[:, icl, :],
                                    lhsT=lhsT_str,
                                    rhs=hp_all[:, jb, hd, :],
                                    start=(jb == 0),
                                    stop=(jb == NB - 1),
                                )
                        gs = slice(4 * grp, 4 * grp + 4)
                        nc.vector.reciprocal(
                            out=rz[:, gs], in_=po[:, :, FO : FO + 1]
                        )
                        rzb = bass.AP(
                            tensor=rz.tensor, offset=rz[:, gs, :].offset,
                            ap=[list(rz.ap[0]), [rz.ap[1][0], 4], [0, FO]],
                        )
                        nc.vector.tensor_mul(
                            out=o_sb[:, gs, :], in0=po[:, :, 0:FO], in1=rzb
                        )
                        nc.sync.dma_start(
                            out=out_re[:, gs], in_=o_sb[:, gs]
                        )

                prev_p = None
                for hd in range(H):
                    p_new = emit_scores(hd)
                    if prev_p is not None:
                        emit_out(hd - 1, prev_p)
                    prev_p = p_new
                emit_out(H - 1, prev_p)
    nc.finalize()
    return nc


_NC_CACHE = None
TRACE = False
LAST_RESULT = None


def kernel(h, adj, w, a_src, a_dst, bias):
    global _NC_CACHE
    from concourse.bass_utils import run_bass_kernel_spmd

    if _NC_CACHE is None:
        _NC_CACHE = build_bass()
    nc = _NC_CACHE

    h = np.ascontiguousarray(np.asarray(h, dtype=np.float32))
    adj_u8 = np.ascontiguousarray(np.asarray(adj).astype(np.uint8))
    w = np.ascontiguousarray(np.asarray(w, dtype=np.float32))
    a_src2 = np.ascontiguousarray(np.asarray(a_src, dtype=np.float32)[..., 0])
    a_dst2 = np.ascontiguousarray(np.asarray(a_dst, dtype=np.float32)[..., 0])
    bias = np.ascontiguousarray(np.asarray(bias, dtype=np.float32))

    in_maps = [
        {
            "h": h[b],
            "adj": adj_u8[b],
            "w": w,
            "a_src": a_src2,
            "a_dst": a_dst2,
            "bias": bias,
        }
        for b in range(BS)
    ]
    res = run_bass_kernel_spmd(
        nc, in_maps, core_ids=list(range(BS)), trace=TRACE,
        trace_cores=list(range(BS)) if TRACE else None,
    )
    if TRACE:
        global LAST_RESULT
        LAST_RESULT = res
    out = np.stack([r["out"] for r in res.results], axis=0)
    return out.astype(np.float32)


# revision 67
# speedup vs baseline: 1.0330x; 1.0119x over previous
"""BatchMultiHeadGraphAttention kernel for TRN2 (8 NeuronCores).

Reference computation (per graph b):
  h_prime = h @ w[head]                 [n, fo] per head
  t = tanh(h_prime)
  src[n] = t @ a_src[head];  dst[n] = t @ a_dst[head]
  s[i, j] = leaky_relu(src[i] + dst[j], 0.2)
  s masked where ~(adj | I); softmax over j; out = p @ h_prime + bias

Sharding: data-parallel over batch - one graph per core (BS=8, 8 cores).

Hybrid per-(head, jb) tile paths, scores transposed (p[j, i]) so the
output matmul contracts over the partition axis:
  - A-path (ACT-heavy, 4 tiles/head): PE matmuls build
    s = dst_j + src_i - 144*invalid in PSUM (two k=1 outer products from
    single-row scatter tiles + an identity matmul folding the additive
    mask), then ACT Prelu + ACT Exp. exp(s - 144) ~ 0 masks invalid edges.
  - V-path (DVE-heavy, 4 tiles/head): rank-1 factorization of the
    reference nonlinearity,
       exp(leaky(x)) = max(exp(x), exp(0.2 x)) = max(E_i*F_j, E~_i*F~_j)
    with E=exp(src), F=exp(dst), E~=exp(0.2 src), F~=exp(0.2 dst): two
    DVE tensor_scalar 4x passes (replicated E rows x per-partition F
    scalars), a DVE tensor max, and a multiplicative {0,1} mask - no N*N
    exp, no PE score streams, no PSUM. Pool/GPSIMD has no ALU ops on TRN2
    (DMA/copy/memset only), so these stay on DVE; the A/V split balances
    ACT vs DVE occupancy (~84 us each).
  - src/dst rows per head pair via one [8, N] PE matmul against weight
    columns (src, dst, and 0.2-scaled copies), one ACT Exp gives all of
    E/F/E~/F~; rows go through a DRAM scratch and come back as
    partition-replicated [128, 2, N] tiles via HWDGE broadcast loads
    (stride-0 leading dim). Per-partition F columns via 8-row PE
    transposes. Raw src/dst rows are DMA-scattered into per-pair
    single-row tiles (no init needed, no cross-pair WAW serialization).
  - adj loads via casting SWDGE DMAs (u8 -> bf16 {0,1}, 8x128
    descriptors to keep the 1024-desc SWDGE ring fluid), DMA-transposed
    on the sync HWDGE queue interleaved with the pair-loop stores so the
    in-order SEQ reaches each as its data lands; diag forced valid with
    an identity max; additive A-mask derived as 144v-144.
  - bias folded into hp via an extra k=1 PE matmul (ones x bias-row);
    softmax normalization makes (sum p*(hp+bias))/sum p = out + bias
    exact; the hp ones-column yields the softmax denominator inside the
    output matmul; out stores via HWDGE on the SP queue.
  - phase B software-pipelined one head deep; emission order is tuned
    against the in-order per-engine SEQ FIFOs (products -> maxes ->
    masks batches; pair heads/tails staggered).
"""

import sys

import numpy as np

try:
    import concourse.bass  # noqa: F401
except ImportError:
    sys.path.insert(0, "/opt/trn_rl_repo")

BS, N, H, FI, FO = 8, 1024, 8, 256, 64
P = 128
NB = N // P     # 8 node blocks
FC = FI // P    # 2 f_in chunks
HP = H // 2     # head pairs
ALPHA = 0.2
BIG = 144.0

# per-head A-path jb positions (4 per head: 32 A-tiles / 32 V-tiles).
# Pool has no ALU ops on TRN2 (DMA/copy/memset only), so all V-path
# max/min ops run on DVE; the A/V split balances ACT vs DVE.
_A_COUNT = [4, 4, 4, 4, 4, 4, 4, 4]
A_JB = {
    hd: set(((hd + t) % NB) for t in range(_A_COUNT[hd]))
    for hd in range(H)
}
def _v_eng(kv):
    return "D", "D"


def build_bass():
    import concourse.bass as bass
    import concourse.mybir as mybir
    from concourse import bacc
    from concourse.masks import make_identity
    from concourse.tile import TileContext

    f32 = mybir.dt.float32
    bf16 = mybir.dt.bfloat16
    u8 = mybir.dt.uint8
    AF = mybir.ActivationFunctionType
    OP = mybir.AluOpType

    nc = bacc.Bacc(trn_type="TRN2")

    h_d = nc.dram_tensor("h", [N, FI], f32, kind="ExternalInput")
    adj_d = nc.dram_tensor("adj", [N, N], u8, kind="ExternalInput")
    w_d = nc.dram_tensor("w", [H, FI, FO], f32, kind="ExternalInput")
    asrc_d = nc.dram_tensor("a_src", [H, FO], f32, kind="ExternalInput")
    adst_d = nc.dram_tensor("a_dst", [H, FO], f32, kind="ExternalInput")
    bias_d = nc.dram_tensor("bias", [FO], f32, kind="ExternalInput")
    out_d = nc.dram_tensor("out", [H, N, FO], f32, kind="ExternalOutput")
    # E/E~/F/F~ rows staged through DRAM for partition-replication loads
    scr_d = nc.dram_tensor("scr", [8 * HP, N], bf16, kind="Internal")

    with TileContext(nc) as tc:
        with (
            tc.tile_pool(name="singles", bufs=1) as singles,
            tc.tile_pool(name="temps", bufs=2) as temps,
            tc.tile_pool(name="ppool", bufs=2) as ppool,
            tc.tile_pool(name="spool", bufs=3) as spool,
            tc.tile_pool(name="epool", bufs=4) as epool,
        ):
            # ---------- long-lived tiles ----------
            ident = singles.tile([P, P], f32)
            make_identity(nc, ident)
            identb = singles.tile([P, P], bf16)
            nc.gpsimd.tensor_copy(out=identb, in_=ident)

            hT_sb = singles.tile([P, FC, N], bf16)  # [f_local, fc, n]
            # A-path score operands: per-pair single-row tiles (scatters
            # fully overwrite them - no init, no WAW serialization). Scores
            # use two k=1 matmuls (dst x ones + ones x src) + mask fold.
            dstp, srcp = [], []
            for q in range(HP):
                dstp.append(singles.tile([1, 2, N], bf16, name=f"dstp{q}"))
                srcp.append(singles.tile([1, 2, N], bf16, name=f"srcp{q}"))
            ones_row = singles.tile([1, P], bf16)
            nc.vector.memset(ones_row, 1.0)
            ones1k = singles.tile([1, N], bf16)
            nc.vector.memset(ones1k, 1.0)

            # hp with ones column (col FO); bias folded in via PE matmul
            hp_all = singles.tile([P, NB, H, FO + 1], bf16)
            nc.vector.memset(hp_all[:, :, :, FO : FO + 1], 1.0)

            # vT {0,1} multiplies V-tiles directly; the A-path folds
            # 144*vT via a pre-scaled identity matmul and the -144
            # constant rides the Prelu bias operand - no derived mask
            vT = singles.tile([P, NB, N], bf16)
            identb144 = singles.tile([P, P], bf16)
            nc.vector.tensor_scalar(
                out=identb144, in0=identb, scalar1=BIG, scalar2=None,
                op0=OP.mult,
            )
            neg144col = singles.tile([P, 1], f32)
            nc.vector.memset(neg144col, -BIG)

            # V-path replicated rows: rotating pool of [P, {E,E~}, N]
            # per-partition scalars: [j_local, jb, pair, row8] (f32)
            cols = singles.tile([P, NB, HP, 8], f32)
            erep_t = {}

            # ================= phase A: prep =================
            with (
                tc.tile_pool(name="phA", bufs=1) as phA,
                tc.tile_pool(name="tempA", bufs=4) as tempA,
                tc.tile_pool(name="psumA", bufs=2, space="PSUM") as psA,
                tc.tile_pool(name="psumHP", bufs=1, space="PSUM") as psHP,
                tc.tile_pool(name="psumR", bufs=2, space="PSUM") as psR,
                tc.tile_pool(name="psumD", bufs=1, space="PSUM") as psD,
                tc.tile_pool(name="psumC", bufs=1, space="PSUM") as psC,
            ):
                # w and h first (HWDGE f32 loads - no SWDGE ring space,
                # hardware desc-gen) + on-chip bf16 converts; they gate the
                # tanh/psd chain everything else hangs off
                w_f = phA.tile([P, FC, H, FO], f32)
                w_re = w_d.rearrange("h (c p) o -> p c h o", p=P)
                w_sb = phA.tile([P, FC, H, FO], bf16)
                h_sb = phA.tile([P, NB, FI], f32)
                h_re = h_d.rearrange("(nb p) f -> p nb f", p=P)
                nc.sync.dma_start(out=w_f[:, 0], in_=w_re[:, 0])
                nc.sync.dma_start(out=h_sb[:, 0:4], in_=h_re[:, 0:4])
                nc.vector.tensor_copy(out=w_sb[:, 0], in_=w_f[:, 0])
                nc.sync.dma_start(out=w_f[:, 1], in_=w_re[:, 1])
                nc.sync.dma_start(out=h_sb[:, 4:8], in_=h_re[:, 4:8])
                nc.vector.tensor_copy(out=w_sb[:, 1], in_=w_f[:, 1])
                # adjacency: casting SWDGE DMA u8 -> bf16 {0.0, 1.0};
                # DMA-transposes follow immediately on the scalar HWDGE
                # queue (keeps the sync queue free for scr/erep/out traffic)
                adjb = phA.tile([P, NB, N], bf16)
                adj_re = adj_d.rearrange("(ib p) j -> p ib j", p=P)
                for ib in range(NB):
                    nc.gpsimd.dma_start(
                        out=adjb[:, ib : ib + 1],
                        in_=adj_re[:, ib : ib + 1],
                    )
                for ib in range(NB):
                    nc.sync.dma_start_transpose(
                        out=vT[:, :, ib * P : (ib + 1) * P],
                        in_=adjb[:, ib],
                    )

                # a_src/a_dst: load as [16, 64], transpose to [64, 16]
                a2d = phA.tile([2 * H, FO], f32)
                nc.sync.dma_start(out=a2d[0:H], in_=asrc_d[:, :])
                nc.sync.dma_start(out=a2d[H : 2 * H], in_=adst_d[:, :])

                # bias row replicated over heads, bf16, with 0 in col FO
                bias_f = phA.tile([1, FO], f32)
                nc.sync.dma_start(
                    out=bias_f,
                    in_=bass.AP(tensor=bias_d, offset=0, ap=[[0, 1], [1, FO]]),
                )
                bias_hrow = phA.tile([1, H, FO + 1], bf16)
                nc.vector.memset(bias_hrow, 0.0)
                nc.vector.tensor_copy(
                    out=bias_hrow[:, :, 0:FO],
                    in_=bass.AP(
                        tensor=bias_f.tensor,
                        offset=bias_f.offset,
                        ap=[list(bias_f.ap[0]), [0, H], [1, FO]],
                    ),
                )

                # a2T: [64, 16] weight columns; a2p8 adds 0.2-scaled copies
                pa2 = psR.tile([FO, 2 * H], f32, tag="hpT")
                nc.tensor.transpose(pa2, a2d, ident[0 : 2 * H, 0 : 2 * H])
                a2T = phA.tile([FO, 2 * H], bf16)
                nc.vector.tensor_copy(out=a2T, in_=pa2)
                # paired-head weight cols: a2p8[:, m, q]; head 2q on parts
                # 0-63, head 2q+1 on 64-127; m = (src, dst, src', dst',
                # .2src, .2dst, .2src', .2dst')
                a2p8 = phA.tile([P, 8, HP], bf16)
                nc.vector.memset(a2p8, 0.0)
                for q in range(HP):
                    nc.vector.tensor_copy(
                        out=a2p8[0:FO, 0:2, q],
                        in_=bass.AP(
                            tensor=a2T.tensor, offset=a2T.offset + 2 * q,
                            ap=[list(a2T.ap[0]), [H, 2]],
                        ),
                    )
                    nc.vector.tensor_copy(
                        out=a2p8[FO : 2 * FO, 2:4, q],
                        in_=bass.AP(
                            tensor=a2T.tensor, offset=a2T.offset + 2 * q + 1,
                            ap=[list(a2T.ap[0]), [H, 2]],
                        ),
                    )
                nc.vector.tensor_scalar(
                    out=a2p8[:, 4:8, :], in0=a2p8[:, 0:4, :],
                    scalar1=ALPHA, scalar2=None, op0=OP.mult,
                )

                # hT: h transposed to [f_local, fc, n]
                for nb in range(NB):
                    for fc in range(FC):
                        pt = psA.tile([P, P], f32, tag="tr")
                        nc.tensor.transpose(
                            pt, h_sb[:, nb, fc * P : (fc + 1) * P], ident
                        )
                        nc.scalar.activation(
                            out=hT_sb[:, fc, nb * P : (nb + 1) * P],
                            in_=pt, func=AF.Copy,
                        )

                # ----- src/dst rows via paired transposed h_prime -----
                # psd8 rows: (src2q, dst2q, src2q+1, dst2q+1) then 0.2x
                # copies; one ACT Exp per pair gives (E,F,E',F',E~,...)
                ptc = psC.tile([P, NB, HP, 8], bf16, tag="cols")
                st8s = {}

                def emit_diag(ib):
                    nc.vector.tensor_max(
                        out=vT[:, ib, ib * P : (ib + 1) * P],
                        in0=vT[:, ib, ib * P : (ib + 1) * P],
                        in1=identb,
                    )

                def emit_pair_head(q):
                    # tanh chain -> psd8 -> SBUF st8; the DVE copy alone
                    # frees the PSUM tile so pairs advance at tanh rate
                    tTp = tempA.tile([P, N], bf16, tag="tT")
                    st8 = phA.tile([8, N], bf16, tag=f"st8{q}")
                    for half in range(2):
                        hs = slice(half * 512, (half + 1) * 512)
                        phT = psR.tile([P, 512], f32, tag="hpT")
                        for fc in range(FC):
                            nc.tensor.matmul(
                                phT,
                                lhsT=w_sb[:, fc, 2 * q : 2 * q + 2, :],
                                rhs=hT_sb[:, fc, hs],
                                start=(fc == 0),
                                stop=(fc == FC - 1),
                                skip_group_check=True,
                            )
                        nc.scalar.activation(
                            out=tTp[:, hs], in_=phT, func=AF.Tanh
                        )
                        psd8 = psD.tile([8, 512], f32, tag="sd")
                        nc.tensor.matmul(
                            psd8, lhsT=a2p8[:, :, q], rhs=tTp[:, hs],
                            start=True, stop=True,
                        )
                        nc.scalar.activation(
                            out=st8[:, hs], in_=psd8, func=AF.Copy
                        )
                    st8s[q] = st8

                def emit_pair_tail(q):
                    # rows: (src2q, dst2q, src2q+1, dst2q+1, 0.2x those)
                    st8 = st8s[q]
                    eAll = phA.tile([8, N], bf16, tag=f"eAll{q}")
                    nc.scalar.activation(out=eAll, in_=st8, func=AF.Exp)
                    # to DRAM scratch for broadcast-loads
                    nc.sync.dma_start(out=scr_d[8 * q : 8 * q + 8, :], in_=eAll)
                    # per-partition scalar columns via PE transposes
                    for jb in range(NB):
                        nc.tensor.transpose(
                            ptc[:, jb, q, :],
                            eAll[:, jb * P : (jb + 1) * P],
                            identb[0:8, 0:8],
                        )
                    nc.vector.tensor_copy(
                        out=cols[:, :, q, :], in_=ptc[:, :, q, :]
                    )
                    # broadcast-load E/E~ replicated rows for both heads
                    for r in range(2):
                        hd = 2 * q + r
                        et = epool.tile([P, 2, N], bf16, tag="erep")
                        nc.sync.dma_start(
                            out=et,
                            in_=bass.AP(
                                tensor=scr_d,
                                offset=(8 * q + 2 * r) * N,
                                ap=[[0, P], [4 * N, 2], [1, N]],
                            ),
                        )
                        erep_t[hd] = et
                    # scatter raw rows into A-path operands:
                    # dsts (st8 rows 1,3) -> dstp[q], srcs (0,2) -> srcp[q]
                    nc.gpsimd.dma_start(
                        out=dstp[q][0:1, :, :],
                        in_=bass.AP(
                            tensor=st8.tensor, offset=st8.offset + N,
                            ap=[[2 * N, 2], [1, N]],
                        ),
                    )
                    nc.gpsimd.dma_start(
                        out=srcp[q][0:1, :, :],
                        in_=bass.AP(
                            tensor=st8.tensor, offset=st8.offset,
                            ap=[[2 * N, 2], [1, N]],
                        ),
                    )
                for q in range(HP):
                    emit_pair_head(q)
                    if q >= 1:
                        emit_pair_tail(q - 1)
                emit_pair_tail(HP - 1)
                # adj diag forcing + additive-mask derivation AFTER the pair
                # loop: keeps the DVE FIFO clear of transpose-gated work
                # while the pair chain drains
                for ib in range(NB):
                    emit_diag(ib)

                # ----- hp (+ones col, bias via extra matmul) -----
                for nb in range(NB):
                    php = psHP.tile([P, H, FO], f32, tag="hp")
                    nc.tensor.matmul(
                        php,
                        lhsT=ones_row,
                        rhs=bias_hrow[:, :, 0:FO],
                        start=True, stop=False,
                    )
                    for fc in range(FC):
                        nc.tensor.matmul(
                            php,
                            lhsT=hT_sb[:, fc, nb * P : (nb + 1) * P],
                            rhs=w_sb[:, fc],
                            start=False,
                            stop=(fc == FC - 1),
                        )
                    nc.vector.tensor_copy(out=hp_all[:, nb, :, 0:FO], in_=php)

            # ================= phase B: attention =================
            with (
                tc.tile_pool(name="psumS", bufs=3, space="PSUM") as psS,
                tc.tile_pool(name="psumO", bufs=1, space="PSUM") as psO,
            ):
                def emit_scores(hd):
                    q, r = hd // 2, hd % 2
                    p_sb = ppool.tile([P, NB, N], bf16, tag="p")
                    a_jbs = [jb for jb in range(NB) if jb in A_JB[hd]]
                    v_jbs = [jb for jb in range(NB) if jb not in A_JB[hd]]
                    # A-path first: PE matmuls + ACT prelu/exp
                    for jb in a_jbs:
                        ps = psS.tile([P, N], f32, tag="spre")
                        for half in range(2):
                            hs = slice(half * 512, (half + 1) * 512)
                            nc.tensor.matmul(
                                ps[:, hs],
                                lhsT=dstp[q][:, r, jb * P : (jb + 1) * P],
                                rhs=ones1k[:, hs],
                                start=True,
                                stop=False,
                                skip_group_check=True,
                            )
                            nc.tensor.matmul(
                                ps[:, hs],
                                lhsT=ones_row,
                                rhs=srcp[q][:, r, hs],
                                start=False,
                                stop=False,
                                skip_group_check=True,
                            )
                            nc.tensor.matmul(
                                ps[:, hs],
                                lhsT=identb144,
                                rhs=vT[:, jb, hs],
                                start=False,
                                stop=True,
                                skip_group_check=True,
                            )
                        nc.scalar.activation(
                            out=ps, in_=ps, func=AF.Prelu, alpha=ALPHA,
                            bias=neg144col,
                        )
                        nc.scalar.activation(
                            out=p_sb[:, jb, :], in_=ps, func=AF.Exp
                        )
                    # V-path: g = E_rep*F_j, g~ = E~_rep*F~_j, max, min-mask
                    # batched: all products, then maxes, then masks, so the
                    # in-order DVE queue never head-of-line blocks on Pool
                    gs, gts, us = {}, {}, {}
                    for jb in v_jbs:
                        g = spool.tile([P, N], bf16, tag="g")
                        nc.vector.tensor_scalar(
                            out=g, in0=erep_t[hd][:, 0, :],
                            scalar1=cols[:, jb, q, 1 + 2 * r : 2 + 2 * r],
                            scalar2=None, op0=OP.mult,
                        )
                        gt = spool.tile([P, N], bf16, tag="gt")
                        nc.vector.tensor_scalar(
                            out=gt, in0=erep_t[hd][:, 1, :],
                            scalar1=cols[:, jb, q, 5 + 2 * r : 6 + 2 * r],
                            scalar2=None, op0=OP.mult,
                        )
                        gs[jb], gts[jb] = g, gt
                    for jb in v_jbs:
                        nc.vector.tensor_max(
                            out=gs[jb], in0=gs[jb], in1=gts[jb]
                        )
                    for jb in v_jbs:
                        nc.vector.tensor_mul(
                            out=p_sb[:, jb, :], in0=gs[jb],
                            in1=vT[:, jb, :],
                        )
                    return p_sb

                def emit_out(hd, p_sb):
                    out_re = out_d[hd].rearrange("(p ic) o -> p ic o", ic=NB)
                    rz = temps.tile([P, NB, 1], f32, tag="rz")
                    o_sb = temps.tile([P, NB, FO], f32, tag="osb")
                    for grp in range(2):
                        po = psO.tile([P, 4, FO + 1], f32,
                                      tag="o2a" if grp == 0 else "o2b")
                        ics = range(4 * grp, 4 * grp + 4)
                        for ic in ics:
                            icl = ic % 4
                            for jb in range(NB):
                                lhsT_str = bass.AP(
                                    tensor=p_sb.tensor,
                                    offset=p_sb[:, jb, ic : ic + 1].offset,
                                    ap=[list(p_sb.ap[0]), [NB, P]],
                                )
                                nc.tensor.matmul(
                                    po[:, icl, :],
                                    lhsT=lhsT_str,
                                    rhs=hp_all[:, jb, hd, :],
                                    start=(jb == 0),
                                    stop=(jb == NB - 1),
                                )
                        gs = slice(4 * grp, 4 * grp + 4)
                        nc.vector.reciprocal(
                            out=rz[:, gs], in_=po[:, :, FO : FO + 1]
                        )
                        rzb = bass.AP(
                            tensor=rz.tensor, offset=rz[:, gs, :].offset,
                            ap=[list(rz.ap[0]), [rz.ap[1][0], 4], [0, FO]],
                        )
                        nc.vector.tensor_mul(
                            out=o_sb[:, gs, :], in0=po[:, :, 0:FO], in1=rzb
                        )
                        nc.sync.dma_start(
                            out=out_re[:, gs], in_=o_sb[:, gs]
                        )

                prev_p = None
                for hd in range(H):
                    p_new = emit_scores(hd)
                    if prev_p is not None:
                        emit_out(hd - 1, prev_p)
                    prev_p = p_new
                emit_out(H - 1, prev_p)
    nc.finalize()
    return nc


_NC_CACHE = None
TRACE = False
LAST_RESULT = None


def kernel(h, adj, w, a_src, a_dst, bias):
    global _NC_CACHE
    from concourse.bass_utils import run_bass_kernel_spmd

    if _NC_CACHE is None:
        _NC_CACHE = build_bass()
    nc = _NC_CACHE

    h = np.ascontiguousarray(np.asarray(h, dtype=np.float32))
    adj_u8 = np.ascontiguousarray(np.asarray(adj).astype(np.uint8))
    w = np.ascontiguousarray(np.asarray(w, dtype=np.float32))
    a_src2 = np.ascontiguousarray(np.asarray(a_src, dtype=np.float32)[..., 0])
    a_dst2 = np.ascontiguousarray(np.asarray(a_dst, dtype=np.float32)[..., 0])
    bias = np.ascontiguousarray(np.asarray(bias, dtype=np.float32))

    in_maps = [
        {
            "h": h[b],
            "adj": adj_u8[b],
            "w": w,
            "a_src": a_src2,
            "a_dst": a_dst2,
            "bias": bias,
        }
        for b in range(BS)
    ]
    res = run_bass_kernel_spmd(
        nc, in_maps, core_ids=list(range(BS)), trace=TRACE,
        trace_cores=list(range(BS)) if TRACE else None,
    )
    if TRACE:
        global LAST_RESULT
        LAST_RESULT = res
    out = np.stack([r["out"] for r in res.results], axis=0)
    return out.astype(np.float32)
